# revision 46
# baseline (speedup 1.0000x reference)
"""Trainium2 Bass kernel for a dense transformer block (attention + LoRA +
MLP + proj), data-parallel over batch across 8 NeuronCores.

Contract: kernel(**inputs) takes the FULL unsharded inputs (numpy arrays,
keys as in reference.setup_inputs()) and returns the FULL [8, 512, 1024]
fp32 output.

Design (per core, one batch element):
  - LoRA is merged into the dense weights on the host (W_eff = W + la@lb,
    exact math since lora_alpha=1) - no LoRA matmuls on device.
  - Weights are pre-tiled in DRAM ([gp, kcp, 128, 2, 256]) so every
    weight DMA is one 128KB contiguous block; bulk weight streams ride
    the sync (hardware-DGE) queue, small latency-critical transfers the
    gpsimd (SWDGE) queue - all queues share the 16 DMA engines.
  - Phase 1: q,k GEMM, 256-col groups alternating the two 1-bank psum
    pools for double-buffering -> qkT channel-major resident.
  - Phase 2 (exp-bound; everything else hides under it): the v-GEMM,
    QK, PV and softmax normalization interleave under the ~32us of
    ACT-engine exp work (exp is 1 elem/cycle/lane, dtype-independent -
    the hard floor of attention).
      * QK packs TWO heads per slot via tile_position row tiling (each
        head contracts over hd=64; a pair's heads live on partition
        halves of the same qkT chunk), and both heads' key-chunk pairs
        land in ONE 4-bank psum tile so a single N=2048 ACT op computes
        each pair-half exp (amortizes the ~290ns ACT op overhead).
      * v-GEMM runs tok-chunk-outer (1 psum bank per unit) from
        resident v-weights; PV for pair p runs two windows later, woven
        round-robin with v units so adjacent matmuls never accumulate
        into the same psum bank.
      * PV keeps the ones-column trick (M=65): masked v rows are zeroed
        so the softmax denominator falls out of the PV matmul for free.
      * Normalization: denominators are DMA-scattered into per-quartet
        [4, 512] tiles, inverted with the fast custom-DVE reciprocal
        (the standard one is ~6.5ns per free element), cast to fp16 and
        broadcast per 128-chunk with a K=4 selection matmul reading the
        quartet tile directly.  The last quartet's chain uses the idle
        ACT engine and sync queue, with xou copies deferred behind it.
  - fc1 starts 3 of its 4 groups before the final normalization lands
    (on the psum banks freed by v/QK); MLP/proj run as 256-col-group
    GEMMs with gelu / +bias+residual / +bias epilogues, proj epilogues
    split across ACT and DVE, outputs streamed per chunk.
  - PSUM budget: QK pool 1x[128,4,S] + 2x[128,S] v-accum + 2x[128,S]
    pv = 8 banks exactly.
  - GEMMs in bf16, psum accumulation fp32, softmax weights bf16, the
    reciprocal path fp32 -> fp16.
"""

import numpy as np

B, S, C = 8, 512, 1024
H, HD, R, HID = 16, 64, 32, 1024
NC3 = 3 * C
NCORES = 8
KC = C // 128          # 8 contraction chunks
VSTRIDE = HD + 1       # v columns per head incl. ones column

_cache = {}


def _get_nc():
    if "nc" in _cache:
        return _cache["nc"]

    from contextlib import ExitStack
    import concourse.tile as tile
    from concourse import bacc, mybir

    f32 = mybir.dt.float32
    f32r = mybir.dt.float32r
    bf16 = mybir.dt.bfloat16
    fp16 = mybir.dt.float16
    AF = mybir.ActivationFunctionType
    ALU = mybir.AluOpType

    nc = bacc.Bacc("TRN2", target_bir_lowering=False, debug=False)

    def din(name, shape, dt=bf16):
        return nc.dram_tensor(name, list(shape), dt, kind="ExternalInput")

    xT_d = din("xT", (C, S))
    mask01_d = din("mask01", (128, 4), f32)
    sel8_d = din("sel8", (4, 512), fp16)
    # weight tiles [gp, kcp, 128, 2, 256]: one DMA = 128KB contiguous
    wqk_d = din("wqk", (8, KC // 2, 128, 2, 256))
    wv_d = din("wv", (2, KC, 128, 512))
    wfc1_d = din("wfc1", (4, KC // 2, 128, 2, 256))
    wfc2_d = din("wfc2", (4, KC // 2, 128, 2, 256))
    wproj_d = din("wproj", (4, KC // 2, 128, 2, 256))
    fc1_b_d = din("fc1_b", (HID,), f32)
    fc2_b_d = din("fc2_b", (C,), f32)
    proj_b_d = din("proj_b", (C,), f32)
    outT_d = nc.dram_tensor("outT", [C, S], f32, kind="ExternalOutput")

    with tile.TileContext(nc) as tc, ExitStack() as ctx:
        resident = ctx.enter_context(tc.tile_pool(name="resident", bufs=1))
        wpool = ctx.enter_context(tc.tile_pool(name="wstream", bufs=24))
        psum2 = ctx.enter_context(tc.tile_pool(name="psum2", bufs=1, space="PSUM"))
        psumv = ctx.enter_context(tc.tile_pool(name="psumv", bufs=2, space="PSUM"))
        psump = ctx.enter_context(tc.tile_pool(name="psump", bufs=2, space="PSUM"))
        expp = ctx.enter_context(tc.tile_pool(name="expp", bufs=3))
        tmpp = ctx.enter_context(tc.tile_pool(name="tmpp", bufs=2))
        outp = ctx.enter_context(tc.tile_pool(name="outp", bufs=2))

        # ---- resident loads (xT split across both DMA queues so the first
        # GEMM group's inputs land fast; bulk prefetch on gpsimd/SWDGE)
        xT = resident.tile([128, KC, S], bf16, name="xT", tag="xT")
        xT_r = xT_d[:].rearrange("(c p) s -> p c s", p=128)
        for kc in range(KC):
            nc.gpsimd.dma_start(xT[:, kc, :], xT_r[:, kc, :])
        mask01 = resident.tile([128, 4], f32, name="mask01", tag="mask01")
        nc.gpsimd.dma_start(mask01[:], mask01_d[:])
        vw = resident.tile([128, 16, 512], bf16, name="vw", tag="vw")
        # n=0 now (needed at window 0); n=1 goes on the sync queue after the
        # q,k weight stream - all queues share the 16 DMA engines, so the
        # 2MB prefetch must not compete with phase-1 weights.
        nc.gpsimd.dma_start(
            vw[:, 0:8, :], wv_d[0].rearrange("k p f -> p k f")
        )
        biases = {}
        for nm, b_d in (("fc1", fc1_b_d), ("fc2", fc2_b_d), ("proj", proj_b_d)):
            biases[nm] = resident.tile(
                [128, KC], f32, name=f"b_{nm}", tag=f"b_{nm}"
            )
            nc.gpsimd.dma_start(
                biases[nm][:], b_d[:].rearrange("(m p) -> p m", p=128)
            )
        sel8 = resident.tile([4, 512], fp16, name="sel8", tag="sel8")
        nc.gpsimd.dma_start(sel8[:], sel8_d[:])

        # ---- other residents
        qkT = resident.tile([128, 16, S], bf16, name="qkT", tag="qkT")
        v = resident.tile([128, 4, H * VSTRIDE], bf16, name="vtok", tag="vtok")
        xou = resident.tile([128, KC, S], bf16, name="xou", tag="xou")
        gT = resident.tile([128, KC, S], bf16, name="gT", tag="gT")
        xo2T = resident.tile([128, KC, S], bf16, name="xo2T", tag="xo2T")
        denq = [
            resident.tile([4, S], f32, name=f"denq{q}", tag=f"denq{q}")
            for q in range(4)
        ]
        recq = [
            resident.tile([4, S], f32, name=f"recq{q}", tag=f"recq{q}")
            for q in range(4)
        ]
        recqh = [
            resident.tile([4, S], fp16, name=f"recqh{q}", tag=f"recqh{q}")
            for q in range(4)
        ]

        # v ones columns (masked): the ones ride along in the PV matmul and
        # produce the softmax denominator for free.
        for h in range(H):
            nc.vector.memset(
                v[:, :, h * VSTRIDE + HD:h * VSTRIDE + HD + 1], 1.0
            )
        for c in range(4):
            ones_cols = v[:, c, :].rearrange("p (h z) -> p h z", z=VSTRIDE)[
                :, :, HD:HD + 1
            ]
            nc.vector.tensor_scalar_mul(ones_cols, ones_cols, mask01[:, c:c + 1])

        # ---- generic 256-col-group GEMM ----------------------------------
        def gemm256(nm, w_d, act, ngp, epilogue):
            # groups alternate the two 1-bank pools for double-buffering
            for gp in range(ngp):
                pool, tag = (psumv, "vacc") if gp % 2 == 0 else (psump, "pv")
                halves = (
                    pool.tile([128, S], f32, name=f"pt_{nm}{gp}a", tag=tag)[:],
                    pool.tile([128, S], f32, name=f"pt_{nm}{gp}b", tag=tag)[:],
                )
                for kcp in range(KC // 2):
                    wt = wpool.tile([128, 2, 256], bf16, tag="w")
                    nc.sync.dma_start(wt[:], w_d[gp, kcp])
                    for k in range(2):
                        kc = 2 * kcp + k
                        for i in range(2):
                            nc.tensor.matmul(
                                halves[i], wt[:, k, 128 * i:128 * (i + 1)],
                                act[:, kc, :],
                                start=(kc == 0), stop=(kc == KC - 1),
                            )
                epilogue(gp, halves)

        # ---- phase 1: q,k GEMM -------------------------------------------
        def qk_epi(gp, halves):
            nc.vector.tensor_copy(qkT[:, 2 * gp, :], halves[0])
            nc.vector.tensor_copy(qkT[:, 2 * gp + 1, :], halves[1])

        gemm256("qk", wqk_d, xT, 8, qk_epi)
        nc.sync.dma_start(
            vw[:, 8:16, :], wv_d[1].rearrange("k p f -> p k f")
        )

        # ---- phase 2: v-GEMM + attention fused under the exp stream ------
        # Windows p=0..9: QK(pair p) for p<8, v-GEMM units per V_SCHED, PV
        # for pair p-2 (a full window of slack between a v unit landing and
        # PV consuming it).  PE matmul streams are woven round-robin so no
        # two adjacent matmuls accumulate into the same PSUM bank.
        V_SCHED = {
            0: [(0, 0), (0, 1)], 1: [(0, 2), (0, 3)],
            2: [(1, 0)], 3: [(1, 1)], 4: [(1, 2)], 5: [(1, 3)],
        }

        def v_unit_thunks(units):
            """Per unit: list of 8 matmul thunks + an epilogue closure."""
            streams, epis = [], []
            for (n, c) in units:
                t = psumv.tile([128, S], f32, name=f"v{n}{c}", tag="vacc")

                def mk(t=t, n=n, c=c, kc=0):
                    return lambda: nc.tensor.matmul(
                        t[:], xT[:, kc, 128 * c:128 * (c + 1)],
                        vw[:, 8 * n + kc, :],
                        start=(kc == 0), stop=(kc == KC - 1),
                    )

                streams.append([mk(kc=kc) for kc in range(KC)])

                def epi(t=t, n=n, c=c):
                    dst = v[
                        :, c, VSTRIDE * 8 * n:VSTRIDE * 8 * (n + 1)
                    ].rearrange("p (h z) -> p h z", z=VSTRIDE)[:, :, 0:HD]
                    src = t[:].rearrange("p (h z) -> p h z", z=HD)
                    nc.vector.tensor_scalar_mul(dst, src, mask01[:, c:c + 1])

                epis.append(epi)
            return streams, epis

        def weave(streams):
            """Emit thunks round-robin across streams (bank interleave)."""
            streams = [list(s) for s in streams if s]
            while streams:
                nxt = []
                for s in streams:
                    s.pop(0)()
                    if s:
                        nxt.append(s)
                streams = nxt

        def den_out(h, pvt):
            tmd = tmpp.tile([128, S], f32, name="tmd", tag="tmpd")
            # the last quartet's chain is latency-critical: copy on the
            # (idle) ACT engine and DMA on the hardware-DGE sync queue
            if h >= 12:
                nc.scalar.copy(tmd[HD:HD + 1, :], pvt[HD:HD + 1, :])
            else:
                nc.vector.tensor_copy(tmd[HD:HD + 1, :], pvt[HD:HD + 1, :])
            q = nc.sync if h >= 12 else nc.gpsimd
            q.dma_start(
                denq[h // 4][h % 4:h % 4 + 1, :], tmd[HD:HD + 1, :]
            )

        def xou_out(h, pvt, cp):
            j, half = h // 2, h % 2
            if half == 0:
                cp(xou[0:64, j, :], pvt[0:HD, :])
            else:
                tmb = tmpp.tile([128, S], bf16, name="tmb", tag="tmpb")
                cp(tmb[0:HD, :], pvt[0:HD, :])
                q = nc.sync if h >= 12 else nc.gpsimd
                q.dma_start(xou[64:128, j, :], tmb[0:HD, :])

        def norm_prep(q):
            # DVE reciprocal cost scales with free size (~3.3us for [*,512]);
            # reciprocal_approx_fast is ~5x faster at 18 correct bits.
            # per-quartet tiles keep most of it off the critical path, and
            # the fp16 copies are DMA-assembled into the [8,512] sel rhs
            # (engine ops can't start at partition 4; DMA can).
            nc.vector.reciprocal_approx_fast(recq[q][:], denq[q][:])
            with nc.allow_low_precision(reason="recip broadcast via fp16"):
                nc.vector.tensor_copy(recqh[q][:], recq[q][:])

        def norm_apply(hb, pool, tag):
            # broadcast recip per 128-chunk with a K=8 fp16 selection
            # matmul, scale xou chunks [4hb, 4hb+4) in place.
            for jj in range(4):
                j = hb * 4 + jj
                pn = pool.tile([128, S], f32, name=f"pn{j}", tag=tag)
                nc.tensor.matmul(
                    pn[:], sel8[:, jj * 128:(jj + 1) * 128],
                    recqh[2 * hb + jj // 2][:],
                )
                nc.vector.tensor_mul(xou[:, j, :], xou[:, j, :], pn[:])

        def fc1_part(halves, gp, kcps, start):
            for kcp in kcps:
                wt = wpool.tile([128, 2, 256], bf16, tag="w")
                nc.sync.dma_start(wt[:], wfc1_d[gp, kcp])
                for k in range(2):
                    kc = 2 * kcp + k
                    for i in range(2):
                        nc.tensor.matmul(
                            halves[i], wt[:, k, 128 * i:128 * (i + 1)],
                            xou[:, kc, :],
                            start=(start and kcp == kcps[0] and k == 0),
                            stop=(kc == KC - 1),
                        )

        fc1_pts = {}

        def pv_thunks(pp, pexp):
            pvtA = psump.tile([128, S], f32, name="pvtA", tag="pv")
            pvtB = psump.tile([128, S], f32, name="pvtB", tag="pv")

            def mk(pvt, hh, off, c):
                idx = off + c + 2 * (c // 2)
                return lambda: nc.tensor.matmul(
                    pvt[0:VSTRIDE, :],
                    v[:, c, hh * VSTRIDE:(hh + 1) * VSTRIDE],
                    pexp[:, idx, :], start=(c == 0), stop=(c == 3),
                )

            sA = [mk(pvtA, 2 * pp, 0, c) for c in range(4)]
            sB = [mk(pvtB, 2 * pp + 1, 2, c) for c in range(4)]
            return sA, sB, pvtA, pvtB

        exps = {}
        deferred_xou = []
        for p in range(9):
            vs, vepis = v_unit_thunks(V_SCHED.get(p, []))
            if p < 8:
                # QK chunks 0,1 for both packed heads into ONE 4-bank tile
                # (A-c at [:,ci,:], B-c at [:,2+ci,:]) so a single N=2048
                # ACT op computes the pair-half exp (amortizes op overhead)
                tq1 = psum2.tile([128, 4, S], f32, name=f"tq1_{p}", tag="qk2")
                for ci in range(2):
                    nc.tensor.matmul(
                        tq1[:, ci, :],
                        qkT[0:64, 8 + p, 128 * ci:128 * (ci + 1)],
                        qkT[0:64, p, :], tile_position=(0, 0),
                    )
                    nc.tensor.matmul(
                        tq1[:, 2 + ci, :],
                        qkT[64:128, 8 + p, 128 * ci:128 * (ci + 1)],
                        qkT[64:128, p, :], tile_position=(64, 0),
                    )
                # exp layout per pair: [A0 A1 B0 B1 A2 A3 B2 B3]
                exp_t = expp.tile([128, 8, S], bf16, name="exp_t", tag="exp")
                nc.scalar.activation(
                    exp_t[:, 0:4, :], tq1[:], AF.Exp, scale=0.125
                )
                # first half of the v work
                weave([s[:4] for s in vs])
                tq2 = psum2.tile([128, 4, S], f32, name=f"tq2_{p}", tag="qk2")
                for ci in range(2):
                    c = 2 + ci
                    nc.tensor.matmul(
                        tq2[:, ci, :],
                        qkT[0:64, 8 + p, 128 * c:128 * (c + 1)],
                        qkT[0:64, p, :], tile_position=(0, 0),
                    )
                    nc.tensor.matmul(
                        tq2[:, 2 + ci, :],
                        qkT[64:128, 8 + p, 128 * c:128 * (c + 1)],
                        qkT[64:128, p, :], tile_position=(64, 0),
                    )
                nc.scalar.activation(
                    exp_t[:, 4:8, :], tq2[:], AF.Exp, scale=0.125
                )
                exps[p] = exp_t
            # PV: pair p-2 per window; window 8 drains pairs 6 AND 7
            pairs = [p - 2] if 2 <= p <= 7 else ([6, 7] if p == 8 else [])
            if not pairs:
                weave([s[4:] for s in vs])
            first = True
            for pq in pairs:
                pv = pv_thunks(pq, exps[pq])
                rest = [s[4:] for s in vs] if first else []
                first = False
                weave(rest + [pv[0], pv[1]])
                hA, hB = 2 * pq, 2 * pq + 1
                den_out(hA, pv[2])
                den_out(hB, pv[3])
                if p == 8:
                    deferred_xou.append((hA, pv[2]))
                    deferred_xou.append((hB, pv[3]))
                else:
                    xou_out(hA, pv[2], nc.vector.tensor_copy)
                    xou_out(hB, pv[3], nc.vector.tensor_copy)
            for epi in vepis:
                epi()
            if p == 3:
                norm_prep(0)
            if p == 5:
                norm_prep(1)
            if p == 6:
                norm_apply(0, psumv, "vacc")
            if p == 7:
                norm_prep(2)
                # fc1 group 2 starts on normalized chunks 0-3 via the idle
                # v-accum banks; keeps late-attention windows PE-dense
                fc1_pts[2] = (
                    psumv.tile([128, S], f32, name="pt_fc1_2a", tag="vacc")[:],
                    psumv.tile([128, S], f32, name="pt_fc1_2b", tag="vacc")[:],
                )
                fc1_part(fc1_pts[2], 2, [0, 1], True)
            if p == 8:
                norm_prep(3)
                for h_, pvt_ in deferred_xou:
                    xou_out(h_, pvt_, nc.vector.tensor_copy)
                pt = psum2.tile([128, 4, S], f32, name="pt_fc101", tag="qk2")
                fc1_pts[0] = (pt[:, 0, :], pt[:, 1, :])
                fc1_pts[1] = (pt[:, 2, :], pt[:, 3, :])
                for gp in range(2):
                    fc1_part(fc1_pts[gp], gp, [0, 1], True)

        # ---- MLP fc1 + gelu ----------------------------------------------
        def fc1_epi(gp, pt):
            for i in range(2):
                m = 2 * gp + i
                nc.scalar.activation(
                    gT[:, m, :], pt[:, i, :], AF.Gelu,
                    bias=biases["fc1"][:, m:m + 1],
                )

        norm_apply(1, psump, "pv")
        # kcp-outer so each freshly normalized chunk unblocks all groups
        for kcp in (2, 3):
            for gp in range(3):
                fc1_part(fc1_pts[gp], gp, [kcp], False)
        for gp in range(3):
            for i in range(2):
                m = 2 * gp + i
                nc.scalar.activation(
                    gT[:, m, :], fc1_pts[gp][i], AF.Gelu,
                    bias=biases["fc1"][:, m:m + 1],
                )
        h3 = (
            psump.tile([128, S], f32, name="pt_fc13a", tag="pv")[:],
            psump.tile([128, S], f32, name="pt_fc13b", tag="pv")[:],
        )
        fc1_part(h3, 3, [0, 1, 2, 3], True)
        for i in range(2):
            nc.scalar.activation(
                gT[:, 6 + i, :], h3[i], AF.Gelu,
                bias=biases["fc1"][:, 6 + i:7 + i],
            )

        # ---- MLP fc2 + residual ------------------------------------------
        def fc2_epi(gp, halves):
            for i in range(2):
                m = 2 * gp + i
                nc.vector.scalar_tensor_tensor(
                    xo2T[:, m, :], halves[i], biases["fc2"][:, m:m + 1],
                    xou[:, m, :], op0=ALU.add, op1=ALU.add,
                )

        gemm256("fc2", wfc2_d, gT, 4, fc2_epi)

        # ---- proj ---------------------------------------------------------
        outT_r = outT_d[:].rearrange("(m p) s -> p m s", p=128)

        def proj_epi(gp, halves):
            # split +bias epilogues across ACT and the (idle) DVE so the
            # output drain is not serialized on one engine
            ot = outp.tile([128, 2, S], f32, name=f"ot{gp}", tag="out")
            for i in range(2):
                m = 2 * gp + i
                if i == 0:
                    nc.scalar.activation(
                        ot[:, i, :], halves[i], AF.Identity,
                        bias=biases["proj"][:, m:m + 1],
                    )
                else:
                    nc.vector.tensor_scalar_add(
                        ot[:, i, :], halves[i], biases["proj"][:, m:m + 1]
                    )
                nc.sync.dma_start(outT_r[:, m, :], ot[:, i, :])

        gemm256("proj", wproj_d, xo2T, 4, proj_epi)

    nc.compile()
    _cache["nc"] = nc
    return nc


def _bf16(a):
    import ml_dtypes

    return np.asarray(a, dtype=np.float32).astype(ml_dtypes.bfloat16)


def _tile_w(w, ncols):
    """[C, N] -> [N/ncols, KC, 128, ncols] contiguous DMA tiles (bf16)."""
    cin, n = w.shape
    t = w.reshape(KC, 128, n // ncols, ncols).transpose(2, 0, 1, 3)
    return np.ascontiguousarray(_bf16(t))


def _tile_w2(w, ncols=256):
    """[C, N] -> [N/ncols, KC/2, 128, 2, ncols] 128KB-contiguous DMA tiles."""
    cin, n = w.shape
    t = w.reshape(KC // 2, 2, 128, n // ncols, ncols).transpose(3, 0, 2, 1, 4)
    return np.ascontiguousarray(_bf16(t))


def _make_in_maps(inputs):
    x = np.asarray(inputs["x"], dtype=np.float32)
    mask = np.asarray(inputs["mask"])
    sel8 = np.zeros((4, 512), dtype=np.float16)
    for jj in range(4):
        for p in range(128):
            sel8[2 * (jj % 2) + p // 64, jj * 128 + p] = 1.0

    def merged(nm):
        w = np.asarray(inputs[f"{nm}_w"], dtype=np.float32)
        la = np.asarray(inputs[f"{nm}_la"], dtype=np.float32)
        lb = np.asarray(inputs[f"{nm}_lb"], dtype=np.float32)
        return w + la @ lb

    wqkv = merged("qkv")
    shared = {
        "sel8": sel8,
        "wqk": _tile_w2(wqkv[:, :2 * C]),
        "wv": _tile_w(wqkv[:, 2 * C:], 512),
        "wfc1": _tile_w2(merged("fc1")),
        "wfc2": _tile_w2(merged("fc2")),
        "wproj": _tile_w2(merged("proj")),
    }
    for k in ("proj_b", "fc1_b", "fc2_b"):
        shared[k] = np.ascontiguousarray(inputs[k], dtype=np.float32)
    in_maps = []
    for b in range(NCORES):
        m01 = mask[b, :S].astype(np.float32)          # 1.0 keep / 0.0 drop
        in_maps.append(
            dict(
                shared,
                xT=np.ascontiguousarray(_bf16(x[b].T)),
                mask01=np.ascontiguousarray(m01.reshape(4, 128).T),
            )
        )
    return in_maps


def _run(inputs, trace=False):
    from concourse.bass_utils import run_bass_kernel_spmd

    nc = _get_nc()
    in_maps = _make_in_maps(inputs)
    res = run_bass_kernel_spmd(nc, in_maps, list(range(NCORES)), trace=trace)
    out = np.stack(
        [np.ascontiguousarray(res.results[b]["outT"].T) for b in range(NCORES)]
    )
    return out, res


def kernel(**inputs):
    out, _ = _run(inputs, trace=False)
    return out
